# revision 27
# baseline (speedup 1.0000x reference)
"""Trainium2 Bass kernel: causal attention with 3D (Rodrigues) RoPE.

Sharding: tensor-parallel over heads (2 heads/core on 8 cores) for
QKV projection + RoPE + SDPA, then an AllToAll redistributes attention
outputs so the output projection is sharded over tokens (512/core).

v2 layout/schedule (all matmuls bf16, PSUM f32):
  xT        [128, 12, 512]/chunk  tokens on the free axis (1 DMA/chunk)
  q/k proj  out^T [dims, tok]: 3 M-tiles of 128 rows covering
            [q0 q1 k0 k1] (384 rows), evicted with 6 ACT pieces into
            rawsT [96, 4m, 512] (plane-major rows)
  rope      shift rows via 4 gpsimd SBUF DMAs on the consolidated
            [96, 4, 512] tiles, then 5 DVE ops -> qk_rotT bf16
  v proj    opposite orientation (x^T stationary, w_v moving):
            psum [tok, 2*96], one ACT evict into v_sb [128, g, 2, 128]
            (col 96 = ones for the softmax denominator, 97:128 unused)
  attention S^T [tk=128, tq=512] (exp on ScalarE, P bf16, PV contracts
            keys on partitions -> no transposes); batch-0 attention is
            interleaved into the projection loop (chunk ch projects
            while cl=ch-1 of batch 0 runs attention) so ScalarE's exp
            hides under projection PE time
  o-proj    w_o fully SBUF-resident (prefetched during attention);
            A2A#1 (head 0) overlaps batch-1 head-1 attention, A2A#2
            overlaps the first o-proj half.
"""

import sys

sys.path.insert(0, "/opt/trn_rl_repo")

import numpy as np

D_MODEL, N_HEADS, HEAD_DIM, MAX_POS = 1536, 16, 96, 4096
B, T = 2, 2048
NTOK = B * T                      # 4096
NCORES = 8
HPC = N_HEADS // NCORES           # 2 heads per core
NTRIP = HEAD_DIM // 3             # 32 triplets
KT = D_MODEL // 128               # 12 contraction tiles
NCH = NTOK // 512                 # 8 token chunks of 512
TQC = T // 512                    # 4 query chunks per batch
SCALE = 1.0 / np.sqrt(HEAD_DIM)

_CACHE = {}


def _build_nc():
    import concourse.bass as bass
    import concourse.mybir as mybir
    import concourse.tile as tile
    from concourse import bacc

    f32 = mybir.dt.float32
    bf16 = mybir.dt.bfloat16
    MUL = mybir.AluOpType.mult
    ADD = mybir.AluOpType.add
    CP = mybir.ActivationFunctionType.Copy
    EXP = mybir.ActivationFunctionType.Exp

    nc = bacc.Bacc("TRN2", target_bir_lowering=False, debug=False,
                   enable_asserts=False, num_devices=NCORES)

    xT = nc.dram_tensor("xT", [D_MODEL, NTOK], bf16, kind="ExternalInput").ap()
    wqkT = nc.dram_tensor("wqkT", [D_MODEL, 384], bf16,
                          kind="ExternalInput").ap()
    wvT = nc.dram_tensor("wvT", [D_MODEL, 192], bf16,
                         kind="ExternalInput").ap()
    woT = nc.dram_tensor("woT", [D_MODEL, D_MODEL], bf16,
                         kind="ExternalInput").ap()
    cco = nc.dram_tensor("cco", [96, 3, T], bf16, kind="ExternalInput").ap()
    msk = nc.dram_tensor("msk", [128, 128], bf16, kind="ExternalInput").ap()
    out = nc.dram_tensor("out", [D_MODEL, 512], f32, kind="ExternalOutput").ap()

    x_r = xT.rearrange("(k p) t -> p k t", p=128)        # [128, 12, 4096]
    wqk_r = wqkT.rearrange("(k p) n -> p k n", p=128)    # [128, 12, 384]
    wv_r = wvT.rearrange("(k p) n -> p k n", p=128)      # [128, 12, 192]
    wo_r = woT.rearrange("(k p) d -> p k d", p=128)      # [128, 12, 1536]

    with tile.TileContext(nc) as tc:
        with tc.tile_pool(name="dram", bufs=1, space="DRAM") as dram:
            a2a_in = [dram.tile([NCH, 96, 512], bf16, name=f"a2a_in{h}")
                      for h in range(HPC)]
            a2a_out = [dram.tile([NCH, 96, 512], bf16, name=f"a2a_out{h}")
                       for h in range(HPC)]

            with tc.tile_pool(name="glob", bufs=1) as gp:
                qk_rot = gp.tile([96, NCH, 4, 512], bf16, tag="qkrot")
                v_sb = gp.tile([128, NTOK // 128, HPC, 128], bf16, tag="vsb")
                m_sb = gp.tile([128, 128], bf16, tag="msb")
                wqk_sb = gp.tile([128, KT, 384], bf16, tag="wqk")
                wv_sb = gp.tile([128, KT, 192], bf16, tag="wv")
                wo_sb = gp.tile([128, KT, D_MODEL], bf16, tag="wo")
                partA = gp.tile([128, KT, 512], f32, tag="partA")

                cc_wi = dram.tile([NCORES, 16], bf16, name="cc_wi")
                cc_wo = dram.tile([NCORES, 16], bf16, name="cc_wo")

                # startup: x chunk 0 goes first on the sync queue (it gates
                # the first matmul); weights go on the scalar DGE queue in
                # parallel.  w_o is deferred to mid-loop so its 4.7MB does
                # not steal startup HBM bandwidth.
                nc.scalar.dma_start(wqk_sb[:, :, 0:192], wqk_r[:, :, 0:192])
                nc.scalar.dma_start(wqk_sb[:, :, 192:384],
                                    wqk_r[:, :, 192:384])
                nc.scalar.dma_start(wv_sb[:], wv_r[:])
                # tiny dummy AllToAll: warms up the collective cores/rings
                # long before the real A2As
                nc.gpsimd.collective_compute(
                    "AllToAll", mybir.AluOpType.bypass,
                    replica_groups=[list(range(NCORES))],
                    ins=[cc_wi[:].opt()], outs=[cc_wo[:].opt()])
                # ones column for the softmax-denominator trick
                nc.vector.memset(v_sb[:, :, :, 96:97], 1.0)

                # -------- phase 1: proj + rope, batch-0 attn interleaved ----
                with tc.tile_pool(name="ph1", bufs=2) as p1s, \
                     tc.tile_pool(name="ph1b", bufs=4) as p1b, \
                     tc.tile_pool(name="ph1c", bufs=3) as p1c, \
                     tc.tile_pool(name="ps_qk", bufs=1, space="PSUM") as pqk, \
                     tc.tile_pool(name="ps_v", bufs=1, space="PSUM") as pvv, \
                     tc.tile_pool(name="ps_s1", bufs=2,
                                  space="PSUM") as ps_s1, \
                     tc.tile_pool(name="ps_pv1", bufs=1,
                                  space="PSUM") as ps_pv1:

                    def attention_head_cl(h, b, cl, ps_s, ps_pv, p2, p2b,
                                          pend_hi, gp_after=None):
                        """single-cl attention (loop-interleaved variant)."""
                        pend = []
                        st = {"gp_after": gp_after, "w": None}

                        def emit_pv():
                            ti, tt, lo, pt, pv, ntk = pend.pop(0)
                            g = b * 16 + tt
                            nc.tensor.matmul(
                                pv[:, lo:512],
                                v_sb[:, g, h, :],
                                pt[:, lo:512], start=(ti == 0),
                                stop=(ti == ntk - 1),
                                skip_group_check=True)
                            if ti == ntk - 1:
                                normalize(pv)

                        def normalize(pv):
                            lcp = p2b.tile([1, 512], f32, tag="lcp",
                                           name="lcp")
                            nc.vector.tensor_copy(lcp[:], pv[96:97, :])
                            linv = p2b.tile([1, 512], f32, tag="linv",
                                            name="linv")
                            nc.vector.reciprocal_approx_fast(linv[:], lcp[:])
                            brow = p2b.tile([96, 512], f32, tag="brow",
                                            name="brow")
                            bc = nc.gpsimd.partition_broadcast(brow[:],
                                                               linv[:])
                            if st["gp_after"] is not None:
                                tile.add_dep_helper(
                                    bc.ins, st["gp_after"].ins, sync=False,
                                    reason="gpsimd queue order")
                                st["gp_after"] = None
                            att = p2b.tile([96, 512], bf16, tag="att",
                                           name="att")
                            nc.vector.tensor_tensor(
                                att[:], pv[0:96, :], brow[:], MUL)
                            st["w"] = nc.sync.dma_start(
                                a2a_in[h][b * TQC + cl, :, :], att[:])

                        pv = ps_pv.tile([128, 512], f32, tag="pv",
                                        name="pv")
                        ntk = 4 * cl + 4
                        order = (list(range(4 * cl, ntk)) +
                                 list(range(4 * cl)))
                        for ti, tt in enumerate(order):
                            kch = b * TQC + tt // 4
                            kco = (tt % 4) * 128
                            qch = b * TQC + cl
                            lo = (tt - 4 * cl) * 128 if tt >= 4 * cl else 0
                            sp = ps_s.tile([128, 512], f32, tag="s",
                                           name="sp")
                            nc.tensor.matmul(
                                sp[:, lo:512],
                                qk_rot[:, kch, 2 + h, kco:kco + 128],
                                qk_rot[:, qch, h, lo:512],
                                start=True, stop=True)
                            pt = p2.tile([128, 512], bf16, tag="p",
                                         name="pt")
                            nc.scalar.activation(
                                pt[:, lo:512], sp[:, lo:512], EXP)
                            if tt >= 4 * cl:
                                nc.vector.tensor_tensor(
                                    pt[:, lo:lo + 128],
                                    pt[:, lo:lo + 128],
                                    m_sb[:], MUL)
                            pend.append((ti, tt, lo, pt, pv, ntk))
                            if len(pend) > pend_hi:
                                emit_pv()
                        while pend:
                            emit_pv()
                        return st["w"]

                    def load_chunk(ch):
                        coff = (ch % TQC) * 512
                        xt = p1s.tile([128, KT, 512], bf16, tag="xt",
                                      name="xt")
                        nc.sync.dma_start(
                            xt[:, 0:6, :],
                            x_r[:, 0:6, ch * 512:(ch + 1) * 512])
                        nc.sync.dma_start(
                            xt[:, 6:12, :],
                            x_r[:, 6:12, ch * 512:(ch + 1) * 512])
                        c_sl = p1s.tile([96, 3, 512], bf16, tag="csl",
                                        name="c_sl")
                        nc.sync.dma_start(c_sl[:], cco[:, :, coff:coff + 512])
                        return c_sl, xt

                    cur = load_chunk(0)
                    nc.sync.dma_start(m_sb[:], msk[:])
                    for ch in range(NCH):
                        c_sl, xt = cur
                        if ch + 1 < NCH:
                            cur = load_chunk(ch + 1)
                        # q/k projection: 4 M-tiles of 96 rows (one per
                        # q0/q1/k0/k1) so the psum rows are partition-aligned
                        # with the plane-major rope layout -> the whole
                        # eviction is ONE lane-aligned DVE copy.
                        psq = pqk.tile([96, 4, 512], f32, tag="psq",
                                       name="psq")
                        for m in range(4):
                            for kt in range(KT):
                                nc.tensor.matmul(
                                    psq[:, m, :],
                                    wqk_sb[:, kt, m * 96:(m + 1) * 96],
                                    xt[:, kt, :], start=(kt == 0),
                                    stop=(kt == KT - 1),
                                    skip_group_check=True)
                        raw = p1s.tile([96, 4, 512], bf16, tag="raw",
                                       name="raw")
                        nc.vector.tensor_copy(raw[:], psq[:])
                        # v projection: x^T stationary, w_v moving; two
                        # passes over one psum bank, DVE-evicted (aligned)
                        for half in range(2):
                            psv = pvv.tile([128, 2, 256], f32, tag="psv",
                                           name="psv")
                            for s in range(2):
                                for kt in range(KT):
                                    nc.tensor.matmul(
                                        psv[:, s, 0:192],
                                        xt[:, kt,
                                           (half * 2 + s) * 128:
                                           (half * 2 + s + 1) * 128],
                                        wv_sb[:, kt, :], start=(kt == 0),
                                        stop=(kt == KT - 1),
                                        skip_group_check=True)
                            nc.vector.tensor_copy(
                                v_sb[:, ch * 4 + half * 2:
                                     ch * 4 + half * 2 + 2, :, 0:96],
                                psv[:, :, 0:192].rearrange(
                                    "p s (h d) -> p s h d", h=2))
                        if ch == 2:
                            # w_o prefetch, consumed ~150us later by o-proj
                            nc.gpsimd.dma_start(wo_sb[:, :, 0:768],
                                                wo_r[:, :, 0:768])
                            nc.gpsimd.dma_start(wo_sb[:, :, 768:1536],
                                                wo_r[:, :, 768:1536])
                        if ch == 6:
                            # re-warm the collective engine shortly before
                            # the real A2As (ring warmth decays)
                            nc.gpsimd.collective_compute(
                                "AllToAll", mybir.AluOpType.bypass,
                                replica_groups=[list(range(NCORES))],
                                ins=[cc_wi[:].opt()], outs=[cc_wo[:].opt()])
                        # rope shifts (SBUF->SBUF DMA on gpsimd queue)
                        g1 = p1s.tile([96, 4, 512], bf16, tag="g1",
                                      name="g1")
                        g2 = p1s.tile([96, 4, 512], bf16, tag="g2",
                                      name="g2")
                        nc.gpsimd.dma_start(g1[0:64, :, :], raw[32:96, :, :])
                        nc.gpsimd.dma_start(g1[64:96, :, :], raw[0:32, :, :])
                        nc.gpsimd.dma_start(g2[0:32, :, :], raw[64:96, :, :])
                        nc.gpsimd.dma_start(g2[32:96, :, :], raw[0:64, :, :])
                        # rope: out = c0*raw + c1*g1 + c2*g2
                        dst = qk_rot[:, ch, :, :]
                        cb = [c_sl[:, d, :].unsqueeze(1).broadcast_to(
                            [96, 4, 512]) for d in range(3)]
                        nc.vector.tensor_tensor(dst, raw[:], cb[0], MUL)
                        nc.vector.tensor_tensor(g1[:], g1[:], cb[1], MUL)
                        nc.vector.tensor_tensor(dst, dst, g1[:], ADD)
                        nc.vector.tensor_tensor(g2[:], g2[:], cb[2], MUL)
                        nc.vector.tensor_tensor(dst, dst, g2[:], ADD)
                        # interleave ONLY head 0's attention (lags 1 chunk):
                        # b0 at ch 1..4, b1 at ch 5..7 + cl3 after the loop.
                        # Head 1 runs entirely after A2A#1's trigger so its
                        # ~36us of PE work hides the collective.
                        if 1 <= ch <= 4:
                            attention_head_cl(0, 0, ch - 1, ps_s1,
                                              ps_pv1, p1b, p1c, 1)
                        elif ch >= 5:
                            attention_head_cl(0, 1, ch - 5, ps_s1,
                                              ps_pv1, p1b, p1c, 1)
                    attention_head_cl(0, 1, 3, ps_s1, ps_pv1, p1b, p1c, 1)
                    with tc.high_priority():
                        trig1 = nc.gpsimd.collective_compute(
                            "AllToAll", mybir.AluOpType.bypass,
                            replica_groups=[list(range(NCORES))],
                            ins=[a2a_in[0].opt()],
                            outs=[a2a_out[0].opt()])
                    last_w = None
                    gp_dep = trig1
                    for b1_ in range(B):
                        for cl_ in range(TQC):
                            last_w = attention_head_cl(
                                1, b1_, cl_, ps_s1, ps_pv1, p1b, p1c, 1,
                                gp_after=gp_dep)
                            gp_dep = None
                    with tc.high_priority():
                        nc.gpsimd.collective_compute(
                            "AllToAll", mybir.AluOpType.bypass,
                            replica_groups=[list(range(NCORES))],
                            ins=[a2a_in[1].opt()],
                            outs=[a2a_out[1].opt()])

                # -------- phase 2: o-proj --------
                with tc.tile_pool(name="ph2", bufs=6) as p2, \
                     tc.tile_pool(name="ph2b", bufs=3) as p2b, \
                     tc.tile_pool(name="ph2c", bufs=1) as p2c:

                    def load_att2(half, after=None):
                        flat = a2a_out[half][:].rearrange(
                            "a b c -> (a b) c").rearrange(
                            "(e p) c -> p e c", p=128)
                        att2 = p2c.tile([128, 6, 512], bf16,
                                        tag=f"att2_{half}",
                                        name=f"att2_{half}")
                        ld = nc.sync.dma_start(att2[:], flat)
                        if after is not None:
                            tile.add_dep_helper(
                                ld.ins, after.ins, sync=False,
                                reason="sync queue order")
                        return att2

                    def oproj_group(half, g4, att2, ps_o):
                        pos = [ps_o.tile([128, 512], f32, tag="o",
                                         name=f"po_{half}_{g4}_{i}")
                               for i in range(4)]
                        for et in range(6):
                            for i in range(4):
                                dt_ = g4 * 512 + i * 128
                                nc.tensor.matmul(
                                    pos[i][:],
                                    wo_sb[:, half * 6 + et,
                                          dt_:dt_ + 128],
                                    att2[:, et, :], start=(et == 0),
                                    stop=(et == 5),
                                    skip_group_check=True)
                        for i in range(4):
                            dt_ = g4 * 4 + i
                            if half == 0:
                                nc.vector.tensor_copy(
                                    partA[:, dt_, :], pos[i][:])
                            else:
                                ot = p2b.tile([128, 512], f32, tag="ot",
                                              bufs=3, name="ot")
                                nc.vector.tensor_tensor(
                                    ot[:], pos[i][:],
                                    partA[:, dt_, :], ADD)
                                nc.sync.dma_start(
                                    out[dt_ * 128:(dt_ + 1) * 128, :],
                                    ot[:])

                    att2A = load_att2(0, last_w)
                    with tc.tile_pool(name="ps_o", bufs=5,
                                      space="PSUM") as ps_o:
                        for g4 in range(3):
                            oproj_group(0, g4, att2A, ps_o)
                        att2B = load_att2(1)
                        for g4 in range(3):
                            oproj_group(1, g4, att2B, ps_o)

    nc.compile()
    return nc


def _plane_major(w):
    """Reorder head-dim rows 3k+i -> 32i+k (per 96-row head block)."""
    idx = np.empty(96, dtype=np.int64)
    for i in range(3):
        for k in range(NTRIP):
            idx[32 * i + k] = 3 * k + i
    return w[idx]


def _prep_inputs(x, w_qkv, w_o, Rs):
    import ml_dtypes
    bf = ml_dtypes.bfloat16

    x = np.asarray(x, dtype=np.float32)
    w_qkv = np.asarray(w_qkv, dtype=np.float32)
    w_o = np.asarray(w_o, dtype=np.float32)
    Rs = np.asarray(Rs, dtype=np.float32)

    xT = np.ascontiguousarray(x.reshape(NTOK, D_MODEL).T).astype(bf)

    # rope coefficients, plane-major rows: C[d, delta, t]
    R = Rs[:T]                                   # (T, 32, 3, 3)
    cco = np.empty((96, 3, T), dtype=np.float32)
    for d in range(3):
        for i in range(3):
            cco[32 * i:32 * i + 32, d, :] = R[:, :, i, (i + d) % 3].T
    cco = cco.astype(bf)

    # lower-triangular mask for the mixed 128x128 diagonal block
    j = np.arange(128)[:, None]
    i = np.arange(128)[None, :]
    msk = (j <= i).astype(bf)

    # w_o columns-for-even-heads first, then odd (matches split A2A halves)
    woT = np.ascontiguousarray(w_o.T)            # rows e = h*96+d
    perm = np.concatenate(
        [np.arange(h * 96, (h + 1) * 96) for h in range(0, 16, 2)] +
        [np.arange(h * 96, (h + 1) * 96) for h in range(1, 16, 2)])
    woTp = np.ascontiguousarray(woT[perm]).astype(bf)

    def w_row(s, h):
        base = (s * N_HEADS + h) * HEAD_DIM
        return w_qkv[base:base + HEAD_DIM]

    in_maps = []
    for c in range(NCORES):
        h0, h1 = 2 * c, 2 * c + 1
        wqk = np.concatenate([
            _plane_major(w_row(0, h0)) * SCALE,
            _plane_major(w_row(0, h1)) * SCALE,
            _plane_major(w_row(1, h0)),
            _plane_major(w_row(1, h1)),
        ], axis=0)                               # [384, 1536]
        wv = np.concatenate([w_row(2, h0), w_row(2, h1)],
                            axis=0)              # [192, 1536]
        in_maps.append({
            "xT": xT,
            "wqkT": np.ascontiguousarray(wqk.T).astype(bf),
            "wvT": np.ascontiguousarray(wv.T).astype(bf),
            "woT": woTp,
            "cco": cco, "msk": msk,
        })
    return in_maps


def kernel(x, w_qkv, w_o, Rs):
    from concourse import bass_utils

    if "nc" not in _CACHE:
        _CACHE["nc"] = _build_nc()
    nc = _CACHE["nc"]
    in_maps = _prep_inputs(x, w_qkv, w_o, Rs)
    res = bass_utils.run_bass_kernel_spmd(
        nc, in_maps, core_ids=list(range(NCORES)))
    full_T = np.concatenate([res.results[c]["out"] for c in range(NCORES)],
                            axis=1)              # [1536, 4096]
    return np.ascontiguousarray(full_T.T).reshape(B, T, D_MODEL)


# revision 30
# speedup vs baseline: 1.0260x; 1.0260x over previous
"""Trainium2 Bass kernel: causal attention with 3D (Rodrigues) RoPE.

Sharding: tensor-parallel over heads (2 heads/core on 8 cores) for
QKV projection + RoPE + SDPA, then an AllToAll redistributes attention
outputs so the output projection is sharded over tokens (512/core).

v2 layout/schedule (all matmuls bf16, PSUM f32):
  xT        [128, 12, 512]/chunk  tokens on the free axis (1 DMA/chunk)
  q/k proj  out^T [dims, tok]: 3 M-tiles of 128 rows covering
            [q0 q1 k0 k1] (384 rows), evicted with 6 ACT pieces into
            rawsT [96, 4m, 512] (plane-major rows)
  rope      shift rows via 4 gpsimd SBUF DMAs on the consolidated
            [96, 4, 512] tiles, then 5 DVE ops -> qk_rotT bf16
  v proj    opposite orientation (x^T stationary, w_v moving):
            psum [tok, 2*96], one ACT evict into v_sb [128, g, 2, 128]
            (col 96 = ones for the softmax denominator, 97:128 unused)
  attention S^T [tk=128, tq=512] (exp on ScalarE, P bf16, PV contracts
            keys on partitions -> no transposes); batch-0 attention is
            interleaved into the projection loop (chunk ch projects
            while cl=ch-1 of batch 0 runs attention) so ScalarE's exp
            hides under projection PE time
  o-proj    w_o fully SBUF-resident (prefetched during attention);
            A2A#1 (head 0) overlaps batch-1 head-1 attention, A2A#2
            overlaps the first o-proj half.
"""

import sys

sys.path.insert(0, "/opt/trn_rl_repo")

import numpy as np

D_MODEL, N_HEADS, HEAD_DIM, MAX_POS = 1536, 16, 96, 4096
B, T = 2, 2048
NTOK = B * T                      # 4096
NCORES = 8
HPC = N_HEADS // NCORES           # 2 heads per core
NTRIP = HEAD_DIM // 3             # 32 triplets
KT = D_MODEL // 128               # 12 contraction tiles
NCH = NTOK // 512                 # 8 token chunks of 512
TQC = T // 512                    # 4 query chunks per batch
SCALE = 1.0 / np.sqrt(HEAD_DIM)

_CACHE = {}


def _build_nc():
    import concourse.bass as bass
    import concourse.mybir as mybir
    import concourse.tile as tile
    from concourse import bacc

    f32 = mybir.dt.float32
    bf16 = mybir.dt.bfloat16
    MUL = mybir.AluOpType.mult
    ADD = mybir.AluOpType.add
    CP = mybir.ActivationFunctionType.Copy
    EXP = mybir.ActivationFunctionType.Exp

    nc = bacc.Bacc("TRN2", target_bir_lowering=False, debug=False,
                   enable_asserts=False, num_devices=NCORES)

    xT = nc.dram_tensor("xT", [D_MODEL, NTOK], bf16, kind="ExternalInput").ap()
    wqkT = nc.dram_tensor("wqkT", [D_MODEL, 384], bf16,
                          kind="ExternalInput").ap()
    wvT = nc.dram_tensor("wvT", [D_MODEL, 192], bf16,
                         kind="ExternalInput").ap()
    woT = nc.dram_tensor("woT", [D_MODEL, D_MODEL], bf16,
                         kind="ExternalInput").ap()
    cco = nc.dram_tensor("cco", [96, 3, T], bf16, kind="ExternalInput").ap()
    msk = nc.dram_tensor("msk", [128, 128], bf16, kind="ExternalInput").ap()
    out = nc.dram_tensor("out", [D_MODEL, 512], f32, kind="ExternalOutput").ap()

    x_r = xT.rearrange("(k p) t -> p k t", p=128)        # [128, 12, 4096]
    wqk_r = wqkT.rearrange("(k p) n -> p k n", p=128)    # [128, 12, 384]
    wv_r = wvT.rearrange("(k p) n -> p k n", p=128)      # [128, 12, 192]
    wo_r = woT.rearrange("(k p) d -> p k d", p=128)      # [128, 12, 1536]

    with tile.TileContext(nc) as tc:
        with tc.tile_pool(name="dram", bufs=1, space="DRAM") as dram:
            a2a_in = [dram.tile([NCH, 96, 512], bf16, name=f"a2a_in{h}")
                      for h in range(HPC)]
            a2a_out = [dram.tile([NCH, 96, 512], bf16, name=f"a2a_out{h}")
                       for h in range(HPC)]

            with tc.tile_pool(name="glob", bufs=1) as gp:
                qk_rot = gp.tile([96, NCH, 4, 512], bf16, tag="qkrot")
                v_sb = gp.tile([128, NTOK // 128, HPC, 128], bf16, tag="vsb")
                m_sb = gp.tile([128, 128], bf16, tag="msb")
                wqk_sb = gp.tile([128, KT, 384], bf16, tag="wqk")
                wv_sb = gp.tile([128, KT, 192], bf16, tag="wv")
                wo_sb = gp.tile([128, KT, D_MODEL], bf16, tag="wo")
                partA = gp.tile([128, KT, 512], f32, tag="partA")

                cc_wi = dram.tile([NCORES, 16], bf16, name="cc_wi")
                cc_wo = dram.tile([NCORES, 16], bf16, name="cc_wo")

                # startup: x chunk 0 goes first on the sync queue (it gates
                # the first matmul); weights go on the scalar DGE queue in
                # parallel.  w_o is deferred to mid-loop so its 4.7MB does
                # not steal startup HBM bandwidth.
                nc.scalar.dma_start(wqk_sb[:, :, 0:192], wqk_r[:, :, 0:192])
                nc.scalar.dma_start(wqk_sb[:, :, 192:384],
                                    wqk_r[:, :, 192:384])
                nc.scalar.dma_start(wv_sb[:], wv_r[:])
                # tiny dummy AllToAll: warms up the collective cores/rings
                # long before the real A2As
                nc.gpsimd.collective_compute(
                    "AllToAll", mybir.AluOpType.bypass,
                    replica_groups=[list(range(NCORES))],
                    ins=[cc_wi[:].opt()], outs=[cc_wo[:].opt()])
                # ones column for the softmax-denominator trick
                nc.vector.memset(v_sb[:, :, :, 96:97], 1.0)

                # -------- phase 1: proj + rope, batch-0 attn interleaved ----
                with tc.tile_pool(name="ph1", bufs=2) as p1s, \
                     tc.tile_pool(name="ph1b", bufs=4) as p1b, \
                     tc.tile_pool(name="ph1c", bufs=3) as p1c, \
                     tc.tile_pool(name="ps_qk", bufs=1, space="PSUM") as pqk, \
                     tc.tile_pool(name="ps_v", bufs=1, space="PSUM") as pvv, \
                     tc.tile_pool(name="ps_s1", bufs=3,
                                  space="PSUM") as ps_s1, \
                     tc.tile_pool(name="ps_pv1", bufs=1,
                                  space="PSUM") as ps_pv1:

                    def attention_head_cl(h, b, cl, ps_s, ps_pv, p2, p2b,
                                          pend_hi, gp_after=None):
                        """single-cl attention (loop-interleaved variant)."""
                        pend = []
                        st = {"gp_after": gp_after, "w": None}

                        def emit_pv():
                            ti, tt, lo, pt, pv, ntk = pend.pop(0)
                            g = b * 16 + tt
                            nc.tensor.matmul(
                                pv[:, lo:512],
                                v_sb[:, g, h, :],
                                pt[:, lo:512], start=(ti == 0),
                                stop=(ti == ntk - 1),
                                skip_group_check=True)
                            if ti == ntk - 1:
                                normalize(pv)

                        def normalize(pv):
                            lcp = p2b.tile([1, 512], f32, tag="lcp",
                                           name="lcp")
                            nc.vector.tensor_copy(lcp[:], pv[96:97, :])
                            linv = p2b.tile([1, 512], f32, tag="linv",
                                            name="linv")
                            nc.vector.reciprocal_approx_fast(linv[:], lcp[:])
                            brow = p2b.tile([96, 512], f32, tag="brow",
                                            name="brow")
                            bc = nc.gpsimd.partition_broadcast(brow[:],
                                                               linv[:])
                            if st["gp_after"] is not None:
                                tile.add_dep_helper(
                                    bc.ins, st["gp_after"].ins, sync=False,
                                    reason="gpsimd queue order")
                                st["gp_after"] = None
                            att = p2b.tile([96, 512], bf16, tag="att",
                                           name="att")
                            nc.vector.tensor_tensor(
                                att[:], pv[0:96, :], brow[:], MUL)
                            st["w"] = nc.sync.dma_start(
                                a2a_in[h][b * TQC + cl, :, :], att[:])

                        pv = ps_pv.tile([128, 512], f32, tag="pv",
                                        name="pv")
                        ntk = 4 * cl + 4
                        order = (list(range(4 * cl, ntk)) +
                                 list(range(4 * cl)))
                        for ti, tt in enumerate(order):
                            kch = b * TQC + tt // 4
                            kco = (tt % 4) * 128
                            qch = b * TQC + cl
                            lo = (tt - 4 * cl) * 128 if tt >= 4 * cl else 0
                            sp = ps_s.tile([128, 512], f32, tag="s",
                                           name="sp")
                            nc.tensor.matmul(
                                sp[:, lo:512],
                                qk_rot[:, kch, 2 + h, kco:kco + 128],
                                qk_rot[:, qch, h, lo:512],
                                start=True, stop=True)
                            pt = p2.tile([128, 512], bf16, tag="p",
                                         name="pt")
                            nc.scalar.activation(
                                pt[:, lo:512], sp[:, lo:512], EXP)
                            if tt >= 4 * cl:
                                nc.vector.tensor_tensor(
                                    pt[:, lo:lo + 128],
                                    pt[:, lo:lo + 128],
                                    m_sb[:], MUL)
                            pend.append((ti, tt, lo, pt, pv, ntk))
                            if len(pend) > pend_hi:
                                emit_pv()
                        while pend:
                            emit_pv()
                        return st["w"]

                    def load_chunk(ch):
                        coff = (ch % TQC) * 512
                        xt = p1s.tile([128, KT, 512], bf16, tag="xt",
                                      name="xt")
                        nc.sync.dma_start(
                            xt[:, 0:6, :],
                            x_r[:, 0:6, ch * 512:(ch + 1) * 512])
                        nc.sync.dma_start(
                            xt[:, 6:12, :],
                            x_r[:, 6:12, ch * 512:(ch + 1) * 512])
                        c_sl = p1s.tile([96, 3, 512], bf16, tag="csl",
                                        name="c_sl")
                        nc.sync.dma_start(c_sl[:], cco[:, :, coff:coff + 512])
                        return c_sl, xt

                    cur = load_chunk(0)
                    nc.sync.dma_start(m_sb[:], msk[:])
                    for ch in range(NCH):
                        c_sl, xt = cur
                        if ch + 1 < NCH:
                            cur = load_chunk(ch + 1)
                        # q/k projection: 3 M-tiles of 128 over [q0 q1 k0 k1]
                        psq = pqk.tile([128, 3, 512], f32, tag="psq",
                                       name="psq")
                        for m in range(3):
                            for kt in range(KT):
                                nc.tensor.matmul(
                                    psq[:, m, :],
                                    wqk_sb[:, kt, m * 128:(m + 1) * 128],
                                    xt[:, kt, :], start=(kt == 0),
                                    stop=(kt == KT - 1),
                                    skip_group_check=True)
                        # evict q/k psum -> rawsT (plane-major rows) on ACT
                        raw = p1s.tile([96, 4, 512], bf16, tag="raw",
                                       name="raw")
                        ev = [
                            (raw[0:96, 0, :], psq[0:96, 0, :]),
                            (raw[0:32, 1, :], psq[96:128, 0, :]),
                            (raw[32:64, 1, :], psq[0:32, 1, :]),
                            (raw[64:96, 1, :], psq[32:64, 1, :]),
                            (raw[0:64, 2, :], psq[64:128, 1, :]),
                            (raw[64:96, 2, :], psq[0:32, 2, :]),
                            (raw[0:32, 3, :], psq[32:64, 2, :]),
                            (raw[32:64, 3, :], psq[64:96, 2, :]),
                            (raw[64:96, 3, :], psq[96:128, 2, :]),
                        ]
                        for dst_ap, src_ap in ev:
                            nc.scalar.activation(dst_ap, src_ap, CP)
                        # v projection: x^T stationary, w_v moving; two
                        # passes over one psum bank, DVE-evicted (aligned)
                        for half in range(2):
                            psv = pvv.tile([128, 2, 256], f32, tag="psv",
                                           name="psv")
                            for s in range(2):
                                for kt in range(KT):
                                    nc.tensor.matmul(
                                        psv[:, s, 0:192],
                                        xt[:, kt,
                                           (half * 2 + s) * 128:
                                           (half * 2 + s + 1) * 128],
                                        wv_sb[:, kt, :], start=(kt == 0),
                                        stop=(kt == KT - 1),
                                        skip_group_check=True)
                            nc.vector.tensor_copy(
                                v_sb[:, ch * 4 + half * 2:
                                     ch * 4 + half * 2 + 2, :, 0:96],
                                psv[:, :, 0:192].rearrange(
                                    "p s (h d) -> p s h d", h=2))
                        if ch == 2:
                            # w_o prefetch, consumed ~150us later by o-proj
                            nc.gpsimd.dma_start(wo_sb[:, :, 0:768],
                                                wo_r[:, :, 0:768])
                            nc.gpsimd.dma_start(wo_sb[:, :, 768:1536],
                                                wo_r[:, :, 768:1536])
                        if ch == 6:
                            # re-warm the collective engine shortly before
                            # the real A2As (ring warmth decays)
                            nc.gpsimd.collective_compute(
                                "AllToAll", mybir.AluOpType.bypass,
                                replica_groups=[list(range(NCORES))],
                                ins=[cc_wi[:].opt()], outs=[cc_wo[:].opt()])
                        # rope shifts (SBUF->SBUF DMA on gpsimd queue)
                        g1 = p1s.tile([96, 4, 512], bf16, tag="g1",
                                      name="g1")
                        g2 = p1s.tile([96, 4, 512], bf16, tag="g2",
                                      name="g2")
                        nc.gpsimd.dma_start(g1[0:64, :, :], raw[32:96, :, :])
                        nc.gpsimd.dma_start(g1[64:96, :, :], raw[0:32, :, :])
                        nc.gpsimd.dma_start(g2[0:32, :, :], raw[64:96, :, :])
                        nc.gpsimd.dma_start(g2[32:96, :, :], raw[0:64, :, :])
                        # rope: out = c0*raw + c1*g1 + c2*g2
                        dst = qk_rot[:, ch, :, :]
                        cb = [c_sl[:, d, :].unsqueeze(1).broadcast_to(
                            [96, 4, 512]) for d in range(3)]
                        nc.vector.tensor_tensor(dst, raw[:], cb[0], MUL)
                        nc.vector.tensor_tensor(g1[:], g1[:], cb[1], MUL)
                        nc.vector.tensor_tensor(dst, dst, g1[:], ADD)
                        nc.vector.tensor_tensor(g2[:], g2[:], cb[2], MUL)
                        nc.vector.tensor_tensor(dst, dst, g2[:], ADD)
                        # interleaved attention (lags 1 chunk): h0/b0 at
                        # ch 1..4, both heads of b1 at ch 5..7.  h1/b0 and
                        # the two cl3 slices run after the loop: h1/b0's
                        # ~25us of PE work hides A2A#1.
                        if 1 <= ch <= 4:
                            attention_head_cl(0, 0, ch - 1, ps_s1,
                                              ps_pv1, p1b, p1c, 2)
                        elif ch >= 5:
                            for h in range(HPC):
                                attention_head_cl(h, 1, ch - 5, ps_s1,
                                                  ps_pv1, p1b, p1c, 2)
                    attention_head_cl(0, 1, 3, ps_s1, ps_pv1, p1b, p1c, 2)
                    with tc.high_priority():
                        trig1 = nc.gpsimd.collective_compute(
                            "AllToAll", mybir.AluOpType.bypass,
                            replica_groups=[list(range(NCORES))],
                            ins=[a2a_in[0].opt()],
                            outs=[a2a_out[0].opt()])
                    attention_head_cl(1, 1, 3, ps_s1, ps_pv1, p1b, p1c, 2,
                                      gp_after=trig1)
                    last_w = None
                    for cl_ in range(TQC):
                        last_w = attention_head_cl(
                            1, 0, cl_, ps_s1, ps_pv1, p1b, p1c, 2)
                    with tc.high_priority():
                        nc.gpsimd.collective_compute(
                            "AllToAll", mybir.AluOpType.bypass,
                            replica_groups=[list(range(NCORES))],
                            ins=[a2a_in[1].opt()],
                            outs=[a2a_out[1].opt()])

                # -------- phase 2: o-proj --------
                with tc.tile_pool(name="ph2", bufs=6) as p2, \
                     tc.tile_pool(name="ph2b", bufs=3) as p2b, \
                     tc.tile_pool(name="ph2c", bufs=1) as p2c:

                    def load_att2(half, after=None):
                        flat = a2a_out[half][:].rearrange(
                            "a b c -> (a b) c").rearrange(
                            "(e p) c -> p e c", p=128)
                        att2 = p2c.tile([128, 6, 512], bf16,
                                        tag=f"att2_{half}",
                                        name=f"att2_{half}")
                        ld = nc.sync.dma_start(att2[:], flat)
                        if after is not None:
                            tile.add_dep_helper(
                                ld.ins, after.ins, sync=False,
                                reason="sync queue order")
                        return att2

                    def oproj_group(half, g4, att2, ps_o):
                        pos = [ps_o.tile([128, 512], f32, tag="o",
                                         name=f"po_{half}_{g4}_{i}")
                               for i in range(4)]
                        for et in range(6):
                            for i in range(4):
                                dt_ = g4 * 512 + i * 128
                                nc.tensor.matmul(
                                    pos[i][:],
                                    wo_sb[:, half * 6 + et,
                                          dt_:dt_ + 128],
                                    att2[:, et, :], start=(et == 0),
                                    stop=(et == 5),
                                    skip_group_check=True)
                        for i in range(4):
                            dt_ = g4 * 4 + i
                            if half == 0:
                                nc.vector.tensor_copy(
                                    partA[:, dt_, :], pos[i][:])
                            else:
                                ot = p2b.tile([128, 512], f32, tag="ot",
                                              bufs=3, name="ot")
                                nc.vector.tensor_tensor(
                                    ot[:], pos[i][:],
                                    partA[:, dt_, :], ADD)
                                nc.sync.dma_start(
                                    out[dt_ * 128:(dt_ + 1) * 128, :],
                                    ot[:])

                    att2A = load_att2(0, last_w)
                    with tc.tile_pool(name="ps_o", bufs=5,
                                      space="PSUM") as ps_o:
                        for g4 in range(3):
                            oproj_group(0, g4, att2A, ps_o)
                        att2B = load_att2(1)
                        for g4 in range(3):
                            oproj_group(1, g4, att2B, ps_o)

    nc.compile()
    return nc


def _plane_major(w):
    """Reorder head-dim rows 3k+i -> 32i+k (per 96-row head block)."""
    idx = np.empty(96, dtype=np.int64)
    for i in range(3):
        for k in range(NTRIP):
            idx[32 * i + k] = 3 * k + i
    return w[idx]


def _prep_inputs(x, w_qkv, w_o, Rs):
    import ml_dtypes
    bf = ml_dtypes.bfloat16

    x = np.asarray(x, dtype=np.float32)
    w_qkv = np.asarray(w_qkv, dtype=np.float32)
    w_o = np.asarray(w_o, dtype=np.float32)
    Rs = np.asarray(Rs, dtype=np.float32)

    xT = np.ascontiguousarray(x.reshape(NTOK, D_MODEL).T).astype(bf)

    # rope coefficients, plane-major rows: C[d, delta, t]
    R = Rs[:T]                                   # (T, 32, 3, 3)
    cco = np.empty((96, 3, T), dtype=np.float32)
    for d in range(3):
        for i in range(3):
            cco[32 * i:32 * i + 32, d, :] = R[:, :, i, (i + d) % 3].T
    cco = cco.astype(bf)

    # lower-triangular mask for the mixed 128x128 diagonal block
    j = np.arange(128)[:, None]
    i = np.arange(128)[None, :]
    msk = (j <= i).astype(bf)

    # w_o columns-for-even-heads first, then odd (matches split A2A halves)
    woT = np.ascontiguousarray(w_o.T)            # rows e = h*96+d
    perm = np.concatenate(
        [np.arange(h * 96, (h + 1) * 96) for h in range(0, 16, 2)] +
        [np.arange(h * 96, (h + 1) * 96) for h in range(1, 16, 2)])
    woTp = np.ascontiguousarray(woT[perm]).astype(bf)

    def w_row(s, h):
        base = (s * N_HEADS + h) * HEAD_DIM
        return w_qkv[base:base + HEAD_DIM]

    in_maps = []
    for c in range(NCORES):
        h0, h1 = 2 * c, 2 * c + 1
        wqk = np.concatenate([
            _plane_major(w_row(0, h0)) * SCALE,
            _plane_major(w_row(0, h1)) * SCALE,
            _plane_major(w_row(1, h0)),
            _plane_major(w_row(1, h1)),
        ], axis=0)                               # [384, 1536]
        wv = np.concatenate([w_row(2, h0), w_row(2, h1)],
                            axis=0)              # [192, 1536]
        in_maps.append({
            "xT": xT,
            "wqkT": np.ascontiguousarray(wqk.T).astype(bf),
            "wvT": np.ascontiguousarray(wv.T).astype(bf),
            "woT": woTp,
            "cco": cco, "msk": msk,
        })
    return in_maps


def kernel(x, w_qkv, w_o, Rs):
    from concourse import bass_utils

    if "nc" not in _CACHE:
        _CACHE["nc"] = _build_nc()
    nc = _CACHE["nc"]
    in_maps = _prep_inputs(x, w_qkv, w_o, Rs)
    res = bass_utils.run_bass_kernel_spmd(
        nc, in_maps, core_ids=list(range(NCORES)))
    full_T = np.concatenate([res.results[c]["out"] for c in range(NCORES)],
                            axis=1)              # [1536, 4096]
    return np.ascontiguousarray(full_T.T).reshape(B, T, D_MODEL)


# revision 33
# speedup vs baseline: 1.0530x; 1.0263x over previous
"""Trainium2 Bass kernel: causal attention with 3D (Rodrigues) RoPE.

Sharding: tensor-parallel over heads (2 heads/core on 8 cores) for
QKV projection + RoPE + SDPA, then an AllToAll redistributes attention
outputs so the output projection is sharded over tokens (512/core).

v2 layout/schedule (all matmuls bf16, PSUM f32):
  xT        [128, 12, 512]/chunk  tokens on the free axis (1 DMA/chunk)
  q/k proj  out^T [dims, tok]: 3 M-tiles of 128 rows covering
            [q0 q1 k0 k1] (384 rows), evicted with 6 ACT pieces into
            rawsT [96, 4m, 512] (plane-major rows)
  rope      shift rows via 4 gpsimd SBUF DMAs on the consolidated
            [96, 4, 512] tiles, then 5 DVE ops -> qk_rotT bf16
  v proj    opposite orientation (x^T stationary, w_v moving):
            psum [tok, 2*96], one ACT evict into v_sb [128, g, 2, 128]
            (col 96 = ones for the softmax denominator, 97:128 unused)
  attention S^T [tk=128, tq=512] (exp on ScalarE, P bf16, PV contracts
            keys on partitions -> no transposes); batch-0 attention is
            interleaved into the projection loop (chunk ch projects
            while cl=ch-1 of batch 0 runs attention) so ScalarE's exp
            hides under projection PE time
  o-proj    w_o fully SBUF-resident (prefetched during attention);
            A2A#1 (head 0) overlaps batch-1 head-1 attention, A2A#2
            overlaps the first o-proj half.
"""

import sys

sys.path.insert(0, "/opt/trn_rl_repo")

import numpy as np

D_MODEL, N_HEADS, HEAD_DIM, MAX_POS = 1536, 16, 96, 4096
B, T = 2, 2048
NTOK = B * T                      # 4096
NCORES = 8
HPC = N_HEADS // NCORES           # 2 heads per core
NTRIP = HEAD_DIM // 3             # 32 triplets
KT = D_MODEL // 128               # 12 contraction tiles
NCH = NTOK // 512                 # 8 token chunks of 512
TQC = T // 512                    # 4 query chunks per batch
SCALE = 1.0 / np.sqrt(HEAD_DIM)

_CACHE = {}


def _build_nc():
    import concourse.bass as bass
    import concourse.mybir as mybir
    import concourse.tile as tile
    from concourse import bacc

    f32 = mybir.dt.float32
    bf16 = mybir.dt.bfloat16
    MUL = mybir.AluOpType.mult
    ADD = mybir.AluOpType.add
    CP = mybir.ActivationFunctionType.Copy
    EXP = mybir.ActivationFunctionType.Exp

    nc = bacc.Bacc("TRN2", target_bir_lowering=False, debug=False,
                   enable_asserts=False, num_devices=NCORES)

    xT = nc.dram_tensor("xT", [D_MODEL, NTOK], bf16, kind="ExternalInput").ap()
    wqkT = nc.dram_tensor("wqkT", [D_MODEL, 384], bf16,
                          kind="ExternalInput").ap()
    wvT = nc.dram_tensor("wvT", [D_MODEL, 192], bf16,
                         kind="ExternalInput").ap()
    woT = nc.dram_tensor("woT", [D_MODEL, D_MODEL], bf16,
                         kind="ExternalInput").ap()
    cco = nc.dram_tensor("cco", [96, 3, T], bf16, kind="ExternalInput").ap()
    msk = nc.dram_tensor("msk", [128, 128], bf16, kind="ExternalInput").ap()
    out = nc.dram_tensor("out", [D_MODEL, 512], f32, kind="ExternalOutput").ap()

    x_r = xT.rearrange("(k p) t -> p k t", p=128)        # [128, 12, 4096]
    wqk_r = wqkT.rearrange("(k p) n -> p k n", p=128)    # [128, 12, 384]
    wv_r = wvT.rearrange("(k p) n -> p k n", p=128)      # [128, 12, 192]
    wo_r = woT.rearrange("(k p) d -> p k d", p=128)      # [128, 12, 1536]

    with tile.TileContext(nc) as tc:
        with tc.tile_pool(name="dram", bufs=1, space="DRAM") as dram:
            a2a_in = [dram.tile([NCH, 96, 512], bf16, name=f"a2a_in{h}")
                      for h in range(HPC)]
            a2a_out = [dram.tile([NCH, 96, 512], bf16, name=f"a2a_out{h}")
                       for h in range(HPC)]

            with tc.tile_pool(name="glob", bufs=1) as gp:
                qk_rot = gp.tile([96, NCH, 4, 512], bf16, tag="qkrot")
                v_sb = gp.tile([128, NTOK // 128, HPC, 128], bf16, tag="vsb")
                m_sb = gp.tile([128, 128], bf16, tag="msb")
                wqk_sb = gp.tile([128, KT, 384], bf16, tag="wqk")
                wv_sb = gp.tile([128, KT, 192], bf16, tag="wv")
                wo_sb = gp.tile([128, KT, D_MODEL], bf16, tag="wo")
                partA = gp.tile([128, KT, 512], f32, tag="partA")

                cc_wi = dram.tile([NCORES, 16], bf16, name="cc_wi")
                cc_wo = dram.tile([NCORES, 16], bf16, name="cc_wo")

                # startup: x chunk 0 goes first on the sync queue (it gates
                # the first matmul); weights go on the scalar DGE queue in
                # parallel.  w_o is deferred to mid-loop so its 4.7MB does
                # not steal startup HBM bandwidth.
                nc.scalar.dma_start(wqk_sb[:, :, 0:192], wqk_r[:, :, 0:192])
                nc.scalar.dma_start(wqk_sb[:, :, 192:384],
                                    wqk_r[:, :, 192:384])
                nc.scalar.dma_start(wv_sb[:], wv_r[:])
                # tiny dummy AllToAll: warms up the collective cores/rings
                # long before the real A2As
                nc.gpsimd.collective_compute(
                    "AllToAll", mybir.AluOpType.bypass,
                    replica_groups=[list(range(NCORES))],
                    ins=[cc_wi[:].opt()], outs=[cc_wo[:].opt()])
                # ones column for the softmax-denominator trick
                nc.vector.memset(v_sb[:, :, :, 96:97], 1.0)

                # -------- phase 1: proj + rope, batch-0 attn interleaved ----
                with tc.tile_pool(name="ph1", bufs=2) as p1s, \
                     tc.tile_pool(name="ph1b", bufs=4) as p1b, \
                     tc.tile_pool(name="ph1c", bufs=3) as p1c, \
                     tc.tile_pool(name="ps_qk", bufs=1, space="PSUM") as pqk, \
                     tc.tile_pool(name="ps_v", bufs=1, space="PSUM") as pvv, \
                     tc.tile_pool(name="ps_s1", bufs=2,
                                  space="PSUM") as ps_s1, \
                     tc.tile_pool(name="ps_pv1", bufs=1,
                                  space="PSUM") as ps_pv1:

                    def attention_head_cl(h, b, cl, ps_s, ps_pv, p2, p2b,
                                          pend_hi, gp_after=None):
                        """single-cl attention (loop-interleaved variant)."""
                        pend = []
                        st = {"gp_after": gp_after, "w": None}

                        def emit_pv():
                            ti, tt, lo, pt, pv, ntk = pend.pop(0)
                            g = b * 16 + tt
                            nc.tensor.matmul(
                                pv[:, lo:512],
                                v_sb[:, g, h, :],
                                pt[:, lo:512], start=(ti == 0),
                                stop=(ti == ntk - 1),
                                skip_group_check=True)
                            if ti == ntk - 1:
                                normalize(pv)

                        def normalize(pv):
                            lcp = p2b.tile([1, 512], f32, tag="lcp",
                                           name="lcp")
                            nc.vector.tensor_copy(lcp[:], pv[96:97, :])
                            linv = p2b.tile([1, 512], f32, tag="linv",
                                            name="linv")
                            nc.vector.reciprocal_approx_fast(linv[:], lcp[:])
                            brow = p2b.tile([96, 512], f32, tag="brow",
                                            name="brow")
                            bc = nc.gpsimd.partition_broadcast(brow[:],
                                                               linv[:])
                            if st["gp_after"] is not None:
                                tile.add_dep_helper(
                                    bc.ins, st["gp_after"].ins, sync=False,
                                    reason="gpsimd queue order")
                                st["gp_after"] = None
                            att = p2b.tile([96, 512], bf16, tag="att",
                                           name="att")
                            nc.vector.tensor_tensor(
                                att[:], pv[0:96, :], brow[:], MUL)
                            st["w"] = nc.sync.dma_start(
                                a2a_in[h][b * TQC + cl, :, :], att[:])

                        pv = ps_pv.tile([128, 512], f32, tag="pv",
                                        name="pv")
                        ntk = 4 * cl + 4
                        order = (list(range(4 * cl, ntk)) +
                                 list(range(4 * cl)))
                        for ti, tt in enumerate(order):
                            kch = b * TQC + tt // 4
                            kco = (tt % 4) * 128
                            qch = b * TQC + cl
                            lo = (tt - 4 * cl) * 128 if tt >= 4 * cl else 0
                            sp = ps_s.tile([128, 512], f32, tag="s",
                                           name="sp")
                            nc.tensor.matmul(
                                sp[:, lo:512],
                                qk_rot[:, kch, 2 + h, kco:kco + 128],
                                qk_rot[:, qch, h, lo:512],
                                start=True, stop=True)
                            pt = p2.tile([128, 512], bf16, tag="p",
                                         name="pt")
                            nc.scalar.activation(
                                pt[:, lo:512], sp[:, lo:512], EXP)
                            if tt >= 4 * cl:
                                nc.vector.tensor_tensor(
                                    pt[:, lo:lo + 128],
                                    pt[:, lo:lo + 128],
                                    m_sb[:], MUL)
                            pend.append((ti, tt, lo, pt, pv, ntk))
                            if len(pend) > pend_hi:
                                emit_pv()
                        while pend:
                            emit_pv()
                        return st["w"]

                    def load_chunk(ch):
                        coff = (ch % TQC) * 512
                        xt = p1s.tile([128, KT, 512], bf16, tag="xt",
                                      name="xt")
                        nc.sync.dma_start(
                            xt[:, 0:6, :],
                            x_r[:, 0:6, ch * 512:(ch + 1) * 512])
                        nc.sync.dma_start(
                            xt[:, 6:12, :],
                            x_r[:, 6:12, ch * 512:(ch + 1) * 512])
                        c_sl = p1s.tile([96, 3, 512], bf16, tag="csl",
                                        name="c_sl")
                        nc.sync.dma_start(c_sl[:], cco[:, :, coff:coff + 512])
                        return c_sl, xt

                    cur = load_chunk(0)
                    nc.sync.dma_start(m_sb[:], msk[:])
                    for ch in range(NCH):
                        c_sl, xt = cur
                        if ch + 1 < NCH:
                            cur = load_chunk(ch + 1)
                        # q/k projection: 4 M-tiles of 96 rows (one per
                        # q0/q1/k0/k1) so the psum rows are partition-aligned
                        # with the plane-major rope layout -> the whole
                        # eviction is ONE lane-aligned DVE copy.
                        psq = pqk.tile([96, 4, 512], f32, tag="psq",
                                       name="psq")
                        for m in range(4):
                            for kt in range(KT):
                                nc.tensor.matmul(
                                    psq[:, m, :],
                                    wqk_sb[:, kt, m * 96:(m + 1) * 96],
                                    xt[:, kt, :], start=(kt == 0),
                                    stop=(kt == KT - 1),
                                    skip_group_check=True)
                        raw = p1s.tile([96, 4, 512], bf16, tag="raw",
                                       name="raw")
                        nc.vector.tensor_copy(raw[:], psq[:])
                        # v projection: x^T stationary, w_v moving; two
                        # passes over one psum bank, DVE-evicted (aligned)
                        for half in range(2):
                            psv = pvv.tile([128, 2, 256], f32, tag="psv",
                                           name="psv")
                            for s in range(2):
                                for kt in range(KT):
                                    nc.tensor.matmul(
                                        psv[:, s, 0:192],
                                        xt[:, kt,
                                           (half * 2 + s) * 128:
                                           (half * 2 + s + 1) * 128],
                                        wv_sb[:, kt, :], start=(kt == 0),
                                        stop=(kt == KT - 1),
                                        skip_group_check=True)
                            nc.vector.tensor_copy(
                                v_sb[:, ch * 4 + half * 2:
                                     ch * 4 + half * 2 + 2, :, 0:96],
                                psv[:, :, 0:192].rearrange(
                                    "p s (h d) -> p s h d", h=2))
                        if ch == 2:
                            # w_o prefetch, consumed ~150us later by o-proj
                            nc.gpsimd.dma_start(wo_sb[:, :, 0:768],
                                                wo_r[:, :, 0:768])
                            nc.gpsimd.dma_start(wo_sb[:, :, 768:1536],
                                                wo_r[:, :, 768:1536])
                        if ch == 6:
                            # re-warm the collective engine shortly before
                            # the real A2As (ring warmth decays)
                            nc.gpsimd.collective_compute(
                                "AllToAll", mybir.AluOpType.bypass,
                                replica_groups=[list(range(NCORES))],
                                ins=[cc_wi[:].opt()], outs=[cc_wo[:].opt()])
                        # rope shifts (SBUF->SBUF DMA on gpsimd queue)
                        g1 = p1s.tile([96, 4, 512], bf16, tag="g1",
                                      name="g1")
                        g2 = p1s.tile([96, 4, 512], bf16, tag="g2",
                                      name="g2")
                        nc.gpsimd.dma_start(g1[0:64, :, :], raw[32:96, :, :])
                        nc.gpsimd.dma_start(g1[64:96, :, :], raw[0:32, :, :])
                        nc.gpsimd.dma_start(g2[0:32, :, :], raw[64:96, :, :])
                        nc.gpsimd.dma_start(g2[32:96, :, :], raw[0:64, :, :])
                        # rope: out = c0*raw + c1*g1 + c2*g2
                        dst = qk_rot[:, ch, :, :]
                        cb = [c_sl[:, d, :].unsqueeze(1).broadcast_to(
                            [96, 4, 512]) for d in range(3)]
                        nc.vector.tensor_tensor(dst, raw[:], cb[0], MUL)
                        nc.vector.tensor_tensor(g1[:], g1[:], cb[1], MUL)
                        nc.vector.tensor_tensor(dst, dst, g1[:], ADD)
                        nc.vector.tensor_tensor(g2[:], g2[:], cb[2], MUL)
                        nc.vector.tensor_tensor(dst, dst, g2[:], ADD)
                        # interleaved attention (lags 1 chunk): batch 0 both
                        # heads at ch 1..4, batch 1 both heads at ch 5..7;
                        # the final b1/cl3 slices + A2A triggers follow the
                        # last chunk immediately.
                        if 1 <= ch <= 4:
                            for h in range(HPC):
                                attention_head_cl(h, 0, ch - 1, ps_s1,
                                                  ps_pv1, p1b, p1c, 1)
                        elif ch >= 5:
                            for h in range(HPC):
                                attention_head_cl(h, 1, ch - 5, ps_s1,
                                                  ps_pv1, p1b, p1c, 1)
                    attention_head_cl(0, 1, 3, ps_s1, ps_pv1, p1b, p1c, 1)
                    with tc.high_priority():
                        trig1 = nc.gpsimd.collective_compute(
                            "AllToAll", mybir.AluOpType.bypass,
                            replica_groups=[list(range(NCORES))],
                            ins=[a2a_in[0].opt()],
                            outs=[a2a_out[0].opt()])
                    last_w = attention_head_cl(1, 1, 3, ps_s1, ps_pv1,
                                               p1b, p1c, 1,
                                               gp_after=trig1)
                    with tc.high_priority():
                        nc.gpsimd.collective_compute(
                            "AllToAll", mybir.AluOpType.bypass,
                            replica_groups=[list(range(NCORES))],
                            ins=[a2a_in[1].opt()],
                            outs=[a2a_out[1].opt()])

                # -------- phase 2: o-proj --------
                with tc.tile_pool(name="ph2", bufs=6) as p2, \
                     tc.tile_pool(name="ph2b", bufs=3) as p2b, \
                     tc.tile_pool(name="ph2c", bufs=1) as p2c:

                    def load_att2(half, after=None):
                        flat = a2a_out[half][:].rearrange(
                            "a b c -> (a b) c").rearrange(
                            "(e p) c -> p e c", p=128)
                        att2 = p2c.tile([128, 6, 512], bf16,
                                        tag=f"att2_{half}",
                                        name=f"att2_{half}")
                        ld = nc.sync.dma_start(att2[:], flat)
                        if after is not None:
                            tile.add_dep_helper(
                                ld.ins, after.ins, sync=False,
                                reason="sync queue order")
                        return att2

                    def oproj_group(half, g4, att2, ps_o):
                        pos = [ps_o.tile([128, 512], f32, tag="o",
                                         name=f"po_{half}_{g4}_{i}")
                               for i in range(4)]
                        for et in range(6):
                            for i in range(4):
                                dt_ = g4 * 512 + i * 128
                                nc.tensor.matmul(
                                    pos[i][:],
                                    wo_sb[:, half * 6 + et,
                                          dt_:dt_ + 128],
                                    att2[:, et, :], start=(et == 0),
                                    stop=(et == 5),
                                    skip_group_check=True)
                        for i in range(4):
                            dt_ = g4 * 4 + i
                            if half == 0:
                                nc.vector.tensor_copy(
                                    partA[:, dt_, :], pos[i][:])
                            else:
                                ot = p2b.tile([128, 512], f32, tag="ot",
                                              bufs=3, name="ot")
                                nc.vector.tensor_tensor(
                                    ot[:], pos[i][:],
                                    partA[:, dt_, :], ADD)
                                nc.sync.dma_start(
                                    out[dt_ * 128:(dt_ + 1) * 128, :],
                                    ot[:])

                    att2A = load_att2(0, last_w)
                    with tc.tile_pool(name="ps_o", bufs=5,
                                      space="PSUM") as ps_o:
                        for g4 in range(3):
                            oproj_group(0, g4, att2A, ps_o)
                        att2B = load_att2(1)
                        for g4 in range(3):
                            oproj_group(1, g4, att2B, ps_o)

    nc.compile()
    return nc


def _plane_major(w):
    """Reorder head-dim rows 3k+i -> 32i+k (per 96-row head block)."""
    idx = np.empty(96, dtype=np.int64)
    for i in range(3):
        for k in range(NTRIP):
            idx[32 * i + k] = 3 * k + i
    return w[idx]


def _prep_inputs(x, w_qkv, w_o, Rs):
    import ml_dtypes
    bf = ml_dtypes.bfloat16

    x = np.asarray(x, dtype=np.float32)
    w_qkv = np.asarray(w_qkv, dtype=np.float32)
    w_o = np.asarray(w_o, dtype=np.float32)
    Rs = np.asarray(Rs, dtype=np.float32)

    xT = np.ascontiguousarray(x.reshape(NTOK, D_MODEL).T).astype(bf)

    # rope coefficients, plane-major rows: C[d, delta, t]
    R = Rs[:T]                                   # (T, 32, 3, 3)
    cco = np.empty((96, 3, T), dtype=np.float32)
    for d in range(3):
        for i in range(3):
            cco[32 * i:32 * i + 32, d, :] = R[:, :, i, (i + d) % 3].T
    cco = cco.astype(bf)

    # lower-triangular mask for the mixed 128x128 diagonal block
    j = np.arange(128)[:, None]
    i = np.arange(128)[None, :]
    msk = (j <= i).astype(bf)

    # w_o columns-for-even-heads first, then odd (matches split A2A halves)
    woT = np.ascontiguousarray(w_o.T)            # rows e = h*96+d
    perm = np.concatenate(
        [np.arange(h * 96, (h + 1) * 96) for h in range(0, 16, 2)] +
        [np.arange(h * 96, (h + 1) * 96) for h in range(1, 16, 2)])
    woTp = np.ascontiguousarray(woT[perm]).astype(bf)

    def w_row(s, h):
        base = (s * N_HEADS + h) * HEAD_DIM
        return w_qkv[base:base + HEAD_DIM]

    in_maps = []
    for c in range(NCORES):
        h0, h1 = 2 * c, 2 * c + 1
        wqk = np.concatenate([
            _plane_major(w_row(0, h0)) * SCALE,
            _plane_major(w_row(0, h1)) * SCALE,
            _plane_major(w_row(1, h0)),
            _plane_major(w_row(1, h1)),
        ], axis=0)                               # [384, 1536]
        wv = np.concatenate([w_row(2, h0), w_row(2, h1)],
                            axis=0)              # [192, 1536]
        in_maps.append({
            "xT": xT,
            "wqkT": np.ascontiguousarray(wqk.T).astype(bf),
            "wvT": np.ascontiguousarray(wv.T).astype(bf),
            "woT": woTp,
            "cco": cco, "msk": msk,
        })
    return in_maps


def kernel(x, w_qkv, w_o, Rs):
    from concourse import bass_utils

    if "nc" not in _CACHE:
        _CACHE["nc"] = _build_nc()
    nc = _CACHE["nc"]
    in_maps = _prep_inputs(x, w_qkv, w_o, Rs)
    res = bass_utils.run_bass_kernel_spmd(
        nc, in_maps, core_ids=list(range(NCORES)))
    full_T = np.concatenate([res.results[c]["out"] for c in range(NCORES)],
                            axis=1)              # [1536, 4096]
    return np.ascontiguousarray(full_T.T).reshape(B, T, D_MODEL)
